# revision 5
# baseline (speedup 1.0000x reference)
"""Trainium2 Bass kernel for causal self-attention with RoPE (B=4, T=2048, C=2048, H=16).

Sharding: 8 cores = 4 batches x 2 head-groups. Core c handles batch c//2 and
heads 8*(c%2)..8*(c%2)+7. Each core computes its QKV slice, head-parallel
attention, and a partial output projection; the host sums the two partials per
batch (tensor-parallel all-reduce) and adds the projection bias.

v3 over v2 (measured 796us vs the 982us v2 baseline by the repeat-slope
method; rel err 3.3e-3 vs the 2e-2 budget):
- Off-diagonal attention in fp8e4 DoubleRow: probs are converted to fp8 in
  key-block-paired tiles (exp bias -2 keeps probs <= e^5, under the TRN e4m3
  max of 240), and both probs@V and the denominator matmul contract 256 keys
  per instruction at double rate. Numerator and denominator share the same
  quantized probs, so fp8 error largely cancels in the softmax normalization.
  Diagonal blocks (the largest weights) stay fp16. The fp8 conversion runs on
  DVE from an fp16 exp result: letting the Exp activation write fp8 directly
  measures ~250us SLOWER on hardware (byte writeback on the activation
  engine); fp16 exp + DVE tensor_copy is fast.
- y never leaves SBUF: attention output overwrites the dead q-half of qk_res
  and the output projection reads it from there.
- out is fp16 (halves the final DMA); host sums partials in f32.
- Stationary-reuse loop orders: V/out/QK projections keep one stationary
  operand across 2-4 moving chunks so the PE skips most weight reloads;
  attention emits scores, then probs@V, then the ones-row sums per chunk.
- Chunked x DMA so the first V-proj matmuls start after ~1/4 of x arrives.
"""

import os
import sys

sys.path.insert(0, "/opt/trn_rl_repo")

import numpy as np

T = 2048
C = 2048
H = 16
DH = 128
B = 4
N_CORES = 8
HLOC = 8          # heads per core
NLOC = HLOC * DH  # 1024 features per core per q/k/v
KT = 16           # 128-row contraction tiles of C
NPAIR = KT // 2
TCH = 512         # token chunk for moving operands
NTC = T // TCH    # 4
SCALE = float(1.0 / np.sqrt(np.float32(DH)))
ROPE_BASE = 10000.0
EXP_BIAS = -2.0   # exp(s*SCALE - 2): keeps probs <= ~e^5 << 240 (TRN e4m3 max)

BUFS_PS1 = int(os.environ.get("BUFS_PS1", "4"))
BUFS_SC = int(os.environ.get("BUFS_SC", "2"))
BUFS_PV = int(os.environ.get("BUFS_PV", "1"))
BUFS_SUM = int(os.environ.get("BUFS_SUM", "1"))
ATTN8 = os.environ.get("ATTN8", "1") == "1"

_CACHE = {}


def _build_bass(repeat=1):
    import concourse.mybir as mybir
    import concourse.tile as tile
    from concourse import bacc

    f32 = mybir.dt.float32
    f16 = mybir.dt.float16
    f8 = mybir.dt.float8e4
    Exp = mybir.ActivationFunctionType.Exp
    DR = mybir.MatmulPerfMode.DoubleRow

    nc = bacc.Bacc()
    xT = nc.declare_dram_parameter("xT", [C, T], f16, isOutput=False)
    # wqk[p, kt, f] = W_qk[128*kt+p, f] (host pre-arranged)
    wqk = nc.declare_dram_parameter("wqk", [128, KT, 2 * NLOC], f16, isOutput=False)
    # bqk[p, n] = b_qk[128*n+p]; bqk_sw is the same rolled by 64 partitions so
    # the rope's cross-half ops read bias and sin table at equal base partitions
    bqk = nc.declare_dram_parameter("bqk", [128, KT], f16, isOutput=False)
    bqk_sw = nc.declare_dram_parameter("bqk_sw", [128, KT], f16, isOutput=False)
    wv = nc.declare_dram_parameter("wv", [C + 1, NLOC], f16, isOutput=False)
    # wp[p, kh, f] = W_proj[group][128*kh+p, f] (host pre-arranged)
    wp = nc.declare_dram_parameter("wp", [128, HLOC, C], f16, isOutput=False)
    cosT = nc.declare_dram_parameter("cosT", [DH, T], f16, isOutput=False)
    sinR = nc.declare_dram_parameter("sinR", [DH, T], f16, isOutput=False)
    dmask = nc.declare_dram_parameter("dmask", [DH, DH], f16, isOutput=False)
    out = nc.declare_dram_parameter("out", [C, T], f16, isOutput=True)

    with tile.TileContext(nc) as tc:
      for rep in range(repeat):
        R = f"r{rep}_" if repeat > 1 else ""
        with (
            tc.tile_pool(name=R + "p_small", bufs=1) as p_small,
            tc.tile_pool(name=R + "p_xt", bufs=1) as p_xt,
            tc.tile_pool(name=R + "p_v", bufs=1) as p_v,
        ):
            dmask_sb = p_small.tile([DH, DH], f16, tag="dmask")
            nc.sync.dma_start(out=dmask_sb[:], in_=dmask[:])
            ones_f16 = p_small.tile([128, 128], f16, tag="ones")
            nc.vector.memset(ones_f16[:], 1.0)
            ones_row = p_small.tile([1, 128], f16, tag="ones_row")
            nc.vector.memset(ones_row[:], 1.0)
            nbias_sb = p_small.tile([128, 1], f32, tag="nbias")
            nc.vector.memset(nbias_sb[:], EXP_BIAS)
            if ATTN8:
                ones8 = p_small.tile([128, 2, 128], f8, tag="ones8")
                nc.vector.tensor_copy(ones8[:, 0, :], ones_f16[:])
                nc.vector.tensor_copy(ones8[:, 1, :], ones_f16[:])

            # x feature-major, resident through the QK projection
            xt = []
            for kt in range(KT):
                t_ = p_xt.tile([128, T], f16, tag=f"xt{kt}", name=f"xt{kt}")
                xt.append(t_)

            # v token-major fp16 (diag blocks) + fp8 key-block pairs (off-diag)
            v_res = [
                p_v.tile([128, NLOC], f16, tag=f"v{i}", name=f"v{i}") for i in range(16)
            ]
            if ATTN8:
                v8 = [
                    p_v.tile([128, 2, NLOC], f8, tag=f"v8_{j}", name=f"v8_{j}")
                    for j in range(NPAIR)
                ]

            # ---------------- V projection (fp16) ----------------
            with (
                tc.tile_pool(name=R + "p_wv", bufs=1) as p_wv,
                tc.tile_pool(name=R + "p_psv", bufs=4, space="PSUM") as p_psv,
            ):
                # pairwise (wv[kt], x[kt] chunk0) arrival order so the first
                # token block's kt-accumulation starts almost immediately
                wvt = []
                for kt in range(KT):
                    w_ = p_wv.tile([128, NLOC], f16, tag=f"wv{kt}", name=f"wv{kt}")
                    wvt.append(w_)
                wvb = p_wv.tile([1, NLOC], f16, tag="wvb")
                for kt in range(KT):
                    nc.sync.dma_start(out=wvt[kt][:], in_=wv[128 * kt : 128 * (kt + 1), :])
                    nc.sync.dma_start(
                        out=xt[kt][:, 0:TCH], in_=xT[128 * kt : 128 * (kt + 1), 0:TCH]
                    )
                nc.sync.dma_start(out=wvb[:], in_=wv[C : C + 1, :])
                for tci in range(1, NTC):
                    csl = slice(TCH * tci, TCH * (tci + 1))
                    for kt in range(KT):
                        nc.sync.dma_start(
                            out=xt[kt][:, csl], in_=xT[128 * kt : 128 * (kt + 1), csl]
                        )

                for tt in range(16):
                    tsl = slice(128 * tt, 128 * (tt + 1))
                    ps0 = p_psv.tile([128, TCH], f32, tag="psv", name=f"psv{tt}_0")
                    ps1 = p_psv.tile([128, TCH], f32, tag="psv", name=f"psv{tt}_1")
                    for kt in range(KT):
                        # one stationary (x block) serves both output halves
                        nc.tensor.matmul(
                            ps0[:], xt[kt][:, tsl], wvt[kt][:, 0:TCH],
                            start=(kt == 0), stop=False,
                        )
                        nc.tensor.matmul(
                            ps1[:], xt[kt][:, tsl], wvt[kt][:, TCH : 2 * TCH],
                            start=(kt == 0), stop=False,
                        )
                    nc.tensor.matmul(
                        ps0[:], ones_row[:], wvb[:, 0:TCH], start=False, stop=True
                    )
                    nc.tensor.matmul(
                        ps1[:], ones_row[:], wvb[:, TCH : 2 * TCH],
                        start=False, stop=True,
                    )
                    nc.scalar.copy(v_res[tt][:, 0:TCH], ps0[:])
                    nc.scalar.copy(v_res[tt][:, TCH : 2 * TCH], ps1[:])
                    if ATTN8:
                        nc.vector.tensor_copy(v8[tt // 2][:, tt % 2, :], v_res[tt][:])

            # ------------- interleaved q/k projection + attention -------------
            with tc.tile_pool(name=R + "p_qk", bufs=1) as p_qk:
                # q/k feature-major, rope'd; attention writes y into the dead
                # q tiles, which the output projection then reads.
                qk_res = [
                    p_qk.tile([128, T], f16, tag=f"qk{n}", name=f"qk{n}")
                    for n in range(16)
                ]
                with (
                    tc.tile_pool(name=R + "p_rope", bufs=1) as p_rope,
                    tc.tile_pool(name=R + "p_w1", bufs=2) as p_w1,
                    tc.tile_pool(name=R + "p_tmp1", bufs=2) as p_tmp1,
                    tc.tile_pool(name=R + "p_probs", bufs=3) as p_probs,
                    tc.tile_pool(name=R + "p_pr8", bufs=7) as p_pr8,
                    tc.tile_pool(name=R + "p_inv", bufs=2) as p_inv,
                    tc.tile_pool(name=R + "p_ps1", bufs=BUFS_PS1, space="PSUM") as p_ps1,
                    tc.tile_pool(name=R + "p_sc", bufs=BUFS_SC, space="PSUM") as p_sc,
                    tc.tile_pool(name=R + "p_pv", bufs=BUFS_PV, space="PSUM") as p_pv,
                    tc.tile_pool(name=R + "p_sum", bufs=BUFS_SUM, space="PSUM") as p_sum,
                ):
                    cos_sb = p_rope.tile([DH, T], f16, tag="cos")
                    sinr_sb = p_rope.tile([DH, T], f16, tag="sinr")
                    nc.sync.dma_start(out=cos_sb[:], in_=cosT[:])
                    nc.sync.dma_start(out=sinr_sb[:], in_=sinR[:])
                    bqk_sb = p_rope.tile([128, KT], f16, tag="bqk")
                    nc.sync.dma_start(out=bqk_sb[:], in_=bqk[:])
                    bqksw_sb = p_rope.tile([128, KT], f16, tag="bqksw")
                    nc.sync.dma_start(out=bqksw_sb[:], in_=bqk_sw[:])
                    Add = mybir.AluOpType.add
                    Mult = mybir.AluOpType.mult

                    def qkproj(n):
                        """Feature tile n (q head n if n<8 else k head n-8) -> rope -> qk_res[n]."""
                        w16 = p_w1.tile([128, KT, 128], f16, tag="w16", name=f"w16_{n}")
                        nc.sync.dma_start(
                            out=w16[:], in_=wqk[:, :, 128 * n : 128 * (n + 1)]
                        )
                        bias = bqk_sb[:, n : n + 1]
                        bias_sw = bqksw_sb[:, n : n + 1]
                        # two tci-groups: ropes for the first half start at the
                        # projection's 50% mark, draining DVE early
                        for grp in range(2):
                            tcis = (2 * grp, 2 * grp + 1)
                            pss = {
                                tci: p_ps1.tile(
                                    [128, TCH], f32, tag="ps1", name=f"psqk{n}_{tci}"
                                )
                                for tci in tcis
                            }
                            for kt in range(KT):
                                for tci in tcis:
                                    # one stationary (weight tile) serves 2 token chunks
                                    nc.tensor.matmul(
                                        pss[tci][:], w16[:, kt, :],
                                        xt[kt][:, TCH * tci : TCH * (tci + 1)],
                                        start=(kt == 0), stop=False,
                                    )
                            for tci in tcis:
                                sl = slice(TCH * tci, TCH * (tci + 1))
                                ps = pss[tci]
                                tmp = p_tmp1.tile([128, TCH], f16, tag="rtmp", name=f"rt{n}_{tci}")
                                nc.vector.scalar_tensor_tensor(
                                    tmp[0:64, :], ps[64:128, :], bias_sw[0:64],
                                    sinr_sb[0:64, sl], Add, Mult,
                                )
                                nc.vector.scalar_tensor_tensor(
                                    tmp[64:128, :], ps[0:64, :], bias_sw[64:128],
                                    sinr_sb[64:128, sl], Add, Mult,
                                )
                                nc.vector.scalar_tensor_tensor(
                                    ps[:], ps[:], bias[:], cos_sb[:, sl], Add, Mult,
                                )
                                nc.vector.tensor_add(qk_res[n][:, sl], ps[:], tmp[:])

                    def attn(h):
                        q_sb, k_sb = qk_res[h], qk_res[8 + h]
                        for tci in range(NTC):
                            pv_ps = p_pv.tile([128, TCH], f32, tag="pv", name=f"pv{h}_{tci}")
                            sum_ps = p_sum.tile([128, TCH], f32, tag="sum", name=f"su{h}_{tci}")
                            qsl_full = slice(TCH * tci, TCH * (tci + 1))
                            npair = 2 * tci if ATTN8 else 0
                            # scores + exp for every key block of this query chunk
                            pr8s = []
                            for j in range(npair):
                                pr8 = p_pr8.tile(
                                    [128, 2, TCH], f8, tag="pr8", name=f"pr8_{h}_{tci}_{j}"
                                )
                                for i in range(2):
                                    si = 2 * j + i
                                    sc_ps = p_sc.tile(
                                        [128, TCH], f32, tag="sc", name=f"sc{h}_{tci}_{si}"
                                    )
                                    nc.tensor.matmul(
                                        sc_ps[:],
                                        k_sb[:, 128 * si : 128 * (si + 1)],
                                        q_sb[:, qsl_full],
                                        start=True, stop=True,
                                    )
                                    ptmp = p_probs.tile(
                                        [128, TCH], f16, tag="pr", name=f"pt{h}_{tci}_{si}"
                                    )
                                    nc.scalar.activation(
                                        ptmp[:], sc_ps[:], Exp,
                                        scale=SCALE, bias=nbias_sb[:],
                                    )
                                    nc.vector.tensor_copy(pr8[:, i, :], ptmp[:])
                                pr8s.append(pr8)
                            probs16 = []
                            diag_lo = 4 * tci if ATTN8 else 0
                            for si in range(diag_lo, 4 * tci + 4):
                                m = si - 4 * tci
                                off = 128 * m if m >= 0 else 0
                                qsl = slice(TCH * tci + off, TCH * (tci + 1))
                                sc_ps = p_sc.tile(
                                    [128, TCH], f32, tag="sc", name=f"scd{h}_{tci}_{si}"
                                )
                                nc.tensor.matmul(
                                    sc_ps[:, off:TCH],
                                    k_sb[:, 128 * si : 128 * (si + 1)],
                                    q_sb[:, qsl],
                                    start=True, stop=True,
                                )
                                probs = p_probs.tile(
                                    [128, TCH], f16, tag="pr", name=f"pr{h}_{tci}_{si}"
                                )
                                nc.scalar.activation(
                                    probs[:, off:TCH], sc_ps[:, off:TCH], Exp,
                                    scale=SCALE, bias=nbias_sb[:],
                                )
                                if m >= 0:
                                    nc.gpsimd.tensor_mul(
                                        probs[:, off : off + 128],
                                        probs[:, off : off + 128],
                                        dmask_sb[:],
                                    )
                                probs16.append((si, off, probs))
                            # probs @ V
                            first = True
                            for j in range(npair):
                                nc.tensor.matmul(
                                    pv_ps[:], v8[j][:, :, 128 * h : 128 * (h + 1)],
                                    pr8s[j][:], start=first, stop=False, perf_mode=DR,
                                )
                                first = False
                            for idx, (si, off, probs) in enumerate(probs16):
                                nc.tensor.matmul(
                                    pv_ps[:, off:TCH],
                                    v_res[si][:, 128 * h : 128 * (h + 1)],
                                    probs[:, off:TCH],
                                    start=first, stop=(idx == len(probs16) - 1),
                                )
                                first = False
                            # denominators: the ones stationary loads once per run
                            first = True
                            for j in range(npair):
                                nc.tensor.matmul(
                                    sum_ps[:], ones8[:], pr8s[j][:],
                                    start=first, stop=False, perf_mode=DR,
                                )
                                first = False
                            for idx, (si, off, probs) in enumerate(probs16):
                                nc.tensor.matmul(
                                    sum_ps[:, off:TCH], ones_f16[:], probs[:, off:TCH],
                                    start=first, stop=(idx == len(probs16) - 1),
                                )
                                first = False
                            inv_sb = p_inv.tile([128, TCH], f16, tag="inv", name=f"inv{h}_{tci}")
                            with nc.allow_low_precision(reason="1/sum in f16; y is stored f16 anyway"):
                                nc.vector.reciprocal(inv_sb[:], sum_ps[:])
                            # y goes into the dead q-chunk of qk_res (read by out proj)
                            nc.vector.tensor_mul(q_sb[:, qsl_full], pv_ps[:], inv_sb[:])

                    # software pipeline: k0,q0, k1,q1, attn0, k2,q2, attn1, ...
                    qkproj(8)
                    qkproj(0)
                    for h in range(HLOC):
                        if h + 1 < HLOC:
                            qkproj(8 + h + 1)
                            qkproj(h + 1)
                        attn(h)

                # ---------------- output projection (y = qk_res[0..7]) ----------------
                with (
                    tc.tile_pool(name=R + "p_wp", bufs=2) as p_wp,
                    tc.tile_pool(name=R + "p_pso", bufs=4, space="PSUM") as p_pso,
                    tc.tile_pool(name=R + "p_osb", bufs=3) as p_osb,
                ):
                    for n in range(16):
                        wpt = p_wp.tile([128, HLOC, 128], f16, tag="wp", name=f"wp{n}")
                        nc.sync.dma_start(
                            out=wpt[:], in_=wp[:, :, 128 * n : 128 * (n + 1)]
                        )
                        pss = [
                            p_pso.tile([128, TCH], f32, tag="pso", name=f"pso{n}_{tci}")
                            for tci in range(NTC)
                        ]
                        for kh in range(HLOC):
                            for tci in range(NTC):
                                sl = slice(TCH * tci, TCH * (tci + 1))
                                nc.tensor.matmul(
                                    pss[tci][:], wpt[:, kh, :], qk_res[kh][:, sl],
                                    start=(kh == 0), stop=(kh == HLOC - 1),
                                )
                        for tci in range(NTC):
                            sl = slice(TCH * tci, TCH * (tci + 1))
                            o_sb = p_osb.tile([128, TCH], f16, tag="osb", name=f"osb{n}_{tci}")
                            nc.scalar.copy(o_sb[:], pss[tci][:])
                            nc.sync.dma_start(
                                out=out[128 * n : 128 * (n + 1), sl], in_=o_sb[:]
                            )

    nc.compile()
    return nc


def _rope_tables():
    inv_freq = 1.0 / (ROPE_BASE ** (np.arange(0, DH, 2, dtype=np.float32) / DH))
    t = np.arange(T, dtype=np.float32)
    freqs = t[:, None] * inv_freq[None, :]
    emb = np.concatenate([freqs, freqs], axis=-1)  # [T, D]
    cos = np.cos(emb).astype(np.float32)
    sin = np.sin(emb).astype(np.float32)
    cosT = np.ascontiguousarray(cos.T)
    sin_rot = np.ascontiguousarray(sin.T)
    sin_rot[:64] = -sin_rot[:64]
    return cosT.astype(np.float16), sin_rot.astype(np.float16)


def make_in_maps(x, W_attn, b_attn, W_proj):
    cosT, sin_rot = _rope_tables()
    dmask = np.where(
        np.arange(DH)[:, None] > np.arange(DH)[None, :],
        np.float16(0.0),
        np.float16(1.0),
    )
    in_maps = []
    for c in range(N_CORES):
        b, g = divmod(c, 2)
        hs = slice(NLOC * g, NLOC * (g + 1))
        xT_t = x[b].T.astype(np.float16)
        wq = W_attn[:, 0 * C : 1 * C][:, hs]
        wk = W_attn[:, 1 * C : 2 * C][:, hs]
        wv_ = W_attn[:, 2 * C : 3 * C][:, hs]
        bq = b_attn[0 * C : 1 * C][hs]
        bk = b_attn[1 * C : 2 * C][hs]
        bv = b_attn[2 * C : 3 * C][hs]
        wqk_full = np.concatenate([wq, wk], axis=1)  # [C, 2*NLOC]
        wqk_t = np.ascontiguousarray(
            wqk_full.reshape(KT, 128, 2 * NLOC).transpose(1, 0, 2)
        ).astype(np.float16)
        bqk16 = np.ascontiguousarray(
            np.concatenate([bq, bk]).reshape(KT, 128).T
        ).astype(np.float16)
        bqk_sw16 = np.ascontiguousarray(np.roll(bqk16, -64, axis=0))
        wv_aug = np.concatenate([wv_, bv[None, :]], axis=0).astype(np.float16)
        wp_t = np.ascontiguousarray(
            W_proj[hs, :].reshape(HLOC, 128, C).transpose(1, 0, 2)
        ).astype(np.float16)
        in_maps.append(
            {
                "xT": np.ascontiguousarray(xT_t),
                "wqk": wqk_t,
                "bqk": np.ascontiguousarray(bqk16),
                "bqk_sw": bqk_sw16,
                "wv": np.ascontiguousarray(wv_aug),
                "wp": wp_t,
                "cosT": cosT,
                "sinR": sin_rot,
                "dmask": dmask,
            }
        )
    return in_maps


def get_nc(repeat=1):
    key = f"nc{repeat}"
    if key not in _CACHE:
        _CACHE[key] = _build_bass(repeat=repeat)
    return _CACHE[key]


def unshard(results, b_proj):
    out = np.empty((B, T, C), dtype=np.float32)
    for b in range(B):
        oT = results[2 * b]["out"].astype(np.float32) + results[2 * b + 1]["out"].astype(
            np.float32
        )
        out[b] = oT.T + b_proj[None, :]
    return out


def _looks_dropped(results):
    """Detect a dropped/partial execution: the donated output buffers start as
    zeros, so a skipped run leaves all-zero 128x512 stripes that a dense
    gaussian output never produces."""
    for r in results:
        o = r["out"]
        blocks = np.abs(o.reshape(C // 128, 128, T // 512, 512)).max(axis=(1, 3))
        if (blocks == 0).any():
            return True
    return False


def kernel(x, W_attn, b_attn, W_proj, b_proj):
    from concourse.bass_utils import run_bass_kernel_spmd

    x = np.asarray(x, dtype=np.float32)
    W_attn = np.asarray(W_attn, dtype=np.float32)
    b_attn = np.asarray(b_attn, dtype=np.float32)
    W_proj = np.asarray(W_proj, dtype=np.float32)
    b_proj = np.asarray(b_proj, dtype=np.float32)

    nc = get_nc()
    in_maps = make_in_maps(x, W_attn, b_attn, W_proj)
    res = run_bass_kernel_spmd(nc, in_maps, list(range(N_CORES)))
    if _looks_dropped(res.results):
        res = run_bass_kernel_spmd(nc, in_maps, list(range(N_CORES)))
    return unshard(res.results, b_proj)
